# revision 7
# baseline (speedup 1.0000x reference)
"""Cross-attention Bass kernel for 8 trn2 NeuronCores.

Sharding: core d handles batch b = d//4 and query rows [(d%4)*1024, (d%4+1)*1024)
of that batch, computing all 8 heads (no collectives). The context is compacted
on the host using the mask (masked rows dropped, zero-padded to a multiple of
128), which preserves softmax semantics exactly while nearly halving the
attention work.

Device dataflow (v2 — k-blocked, engine-balanced):
  - Q^T = Wq^T x^T (bf16, softmax scale folded into the drain), K^T = Wk^T ctx^T
    (bf16), V natural = ctx^T-contracted with Wv (bf16) with a per-head "ones"
    column carrying the valid mask.
  - Scores computed transposed per 2-ktile group: S^T[k, q] in a single 4-bank
    PSUM tile (2 ktiles x 2 heads); one big exp on ScalarE -> P^T bf16.
  - PV in NATURAL orientation: O[q, 65] += P^T-chunk.T @ [V|valid] with bf16
    operands (free dim 65 -> half the PE rows of the O^T orientation).
    Accumulation chains span an 8-ktile super-block in PSUM, then drain-add
    into an SBUF O accumulator on VectorE. The softmax denominator rides along
    as column 64, so normalization is a per-partition scalar multiply - no
    DRAM broadcast round-trips.
  - K/V/ctx^T projection work is split into small units and interleaved into
    the attention group loop (one unit per score group), so the Tensor engine
    never idles while ScalarE exps and vice versa.
  - Epilogue: normalize on VectorE, PE-transpose O, output projection, bias on
    VectorE, store via Pool-engine DMA.

Engine budget per core (cost-model rows): PE ~370k cycles, ACT ~135us (exp),
DVE ~80us, DMA ~47us.
"""
import numpy as np

B, N, M = 2, 4096, 4096
QUERY_DIM, CONTEXT_DIM = 512, 768
H, D = 8, 64
INNER = H * D  # 512
NCORES = 8
N_DEV = (B * N) // NCORES  # 1024 query rows per core
M_PAD_MIN = 256

_compiled = {}


def _build(m_pad):
    from collections import deque

    from concourse import bacc
    import concourse.bass as bass
    import concourse.mybir as mybir
    import concourse.tile as tile
    from concourse.masks import make_identity

    F32 = mybir.dt.float32
    F32R = mybir.dt.float32r
    BF16 = mybir.dt.bfloat16
    AF = mybir.ActivationFunctionType

    KT = m_pad // 128
    SCALE = float(D) ** -0.5
    CQ = QUERY_DIM // 128  # 4
    CC = CONTEXT_DIM // 128  # 6
    CI = INNER // 128  # 4
    NQT = N_DEV // 128  # 8
    QB = 512
    NQB = N_DEV // QB  # 2

    SB = 8  # ktiles per PV-accumulation super-block
    sbs = [(s, min(SB, KT - s)) for s in range(0, KT, SB)]
    # projection j-blocks: pairs of ktiles; fold an odd trailing ktile into a
    # final 3-wide block so every K-projection matmul keeps free dim >= 256
    if KT % 2 == 1 and KT >= 3:
        jbs = [(b, 2) for b in range(0, KT - 3, 2)] + [(KT - 3, 3)]
    else:
        jbs = [(b, min(2, KT - b)) for b in range(0, KT, 2)]
    JW = 384 if (KT % 2 == 1 and KT >= 3) else 256  # widest j-block

    nc = bacc.Bacc()
    xs_d = nc.declare_dram_parameter("xs", [N_DEV, QUERY_DIM], F32, isOutput=False)
    ctx_d = nc.declare_dram_parameter("ctx", [m_pad, CONTEXT_DIM], F32, isOutput=False)
    val_d = nc.declare_dram_parameter("valid", [m_pad], F32, isOutput=False)
    wq_d = nc.declare_dram_parameter("Wq", [QUERY_DIM, INNER], F32, isOutput=False)
    wk_d = nc.declare_dram_parameter("Wk", [CONTEXT_DIM, INNER], F32, isOutput=False)
    wv_d = nc.declare_dram_parameter("Wv", [CONTEXT_DIM, INNER], F32, isOutput=False)
    wo_d = nc.declare_dram_parameter("Wo", [INNER, QUERY_DIM], F32, isOutput=False)
    bo_d = nc.declare_dram_parameter("bo", [QUERY_DIM], F32, isOutput=False)
    out_d = nc.declare_dram_parameter("out", [N_DEV, QUERY_DIM], F32, isOutput=True)

    with tile.TileContext(nc) as tc:
        with (
            tc.tile_pool(name="big", bufs=1) as big,
            tc.tile_pool(name="strm", bufs=2) as strm,
            tc.tile_pool(name="ld", bufs=3) as ld,
            tc.tile_pool(name="ptp", bufs=3) as ptp,
            tc.tile_pool(name="outp", bufs=2) as outp,
            tc.tile_pool(name="nrm", bufs=4) as nrm,
            tc.tile_pool(name="ps_sc", bufs=1, space="PSUM") as ps_sc,
            tc.tile_pool(name="ps_pv", bufs=2, space="PSUM") as ps_pv,
            tc.tile_pool(name="ps_pj", bufs=1, space="PSUM") as ps_pj,
        ):
            # ---- persistent SBUF tiles ----
            qT = big.tile([128, CI, N_DEV], BF16, tag="qT", name="qT")
            kT = big.tile([128, CI, m_pad], BF16, tag="kT", name="kT")
            v2 = [
                big.tile([128, H * 65], BF16, tag=f"v2_{t}", name=f"v2_{t}")
                for t in range(KT)
            ]
            # O accumulator: [q-tile, head, 64 dims + denominator]
            O = big.tile([128, NQT, H, 65], F32R, tag="O", name="O")
            rcp = big.tile([128, NQT, H], F32, tag="rcp", name="rcp")
            wo = big.tile([128, CI, QUERY_DIM], F32R, tag="wo", name="wo")
            bo_bc = big.tile([128, QUERY_DIM], F32, tag="bo", name="bo")
            valid = big.tile([128, KT], F32, tag="valid", name="valid")
            oT = [
                big.tile([128, CI, QB], F32R, tag=f"oT{qb}", name=f"oT{qb}")
                for qb in range(NQB)
            ]
            wk = big.tile([128, CC, INNER], F32R, tag="wk", name="wk")
            wv = big.tile([128, CC, INNER], F32R, tag="wv", name="wv")
            identf = big.tile([128, 128], F32, tag="identf", name="identf")
            ident = big.tile([128, 128], F32R, tag="ident", name="ident")

            # ---- prologue: x^T and Q^T ----
            with tc.tile_pool(name="pro", bufs=1) as pro:
                xs = pro.tile([128, NQT, QUERY_DIM], F32R, tag="xs", name="xs")
                xT = pro.tile([128, CQ, N_DEV], F32R, tag="xT", name="xT")
                wq = pro.tile([128, CQ, INNER], F32R, tag="wq", name="wq")
                # DMA priority order on the sync queue
                nc.gpsimd.dma_start(
                    out=xs[:], in_=xs_d[:].rearrange("(t p) f -> p t f", p=128)
                )
                nc.gpsimd.dma_start(
                    out=wq[:], in_=wq_d[:].rearrange("(o p) f -> p o f", p=128)
                )
                nc.gpsimd.dma_start(
                    out=wk[:], in_=wk_d[:].rearrange("(o p) f -> p o f", p=128)
                )
                nc.gpsimd.dma_start(
                    out=wv[:], in_=wv_d[:].rearrange("(o p) f -> p o f", p=128)
                )
                nc.sync.dma_start(
                    out=valid[:], in_=val_d[:].rearrange("(t p) -> p t", p=128)
                )
                nc.sync.dma_start(
                    out=bo_bc[:],
                    in_=bass.AP(tensor=bo_d, offset=0, ap=[[0, 128], [1, QUERY_DIM]]),
                )
                make_identity(nc, identf[:])
                nc.vector.tensor_copy(ident[:], identf[:])

                # x^T: ping-pong transposes across the two psum pools
                for nt in range(NQT):
                    if nt % 2 == 0:
                        pj = ps_pj.tile([128, 1024], F32R, tag="pj", name="pjx")
                        dst = pj
                    else:
                        pvps = ps_pv.tile([128, 512], F32R, tag="pv", name="pvx")
                        dst = pvps
                    for c in range(CQ):
                        nc.tensor.transpose(
                            dst[:, c * 128 : (c + 1) * 128],
                            xs[:, nt, c * 128 : (c + 1) * 128],
                            ident[:],
                        )
                    nc.vector.tensor_copy(
                        xT[:, :, nt * 128 : (nt + 1) * 128],
                        dst[:, 0 : CQ * 128].rearrange("p (c n) -> p c n", n=128),
                    )

                # Q^T (scale folded into the ACT drain, bf16 out)
                for dc in range(CI):
                    for qf in range(N_DEV // QB):
                        psq = ps_pv.tile([128, 512], F32, tag="pv", name="psq")
                        for c in range(CQ):
                            nc.tensor.matmul(
                                psq[:],
                                wq[:, c, dc * 128 : (dc + 1) * 128],
                                xT[:, c, qf * QB : (qf + 1) * QB],
                                start=(c == 0),
                                stop=(c == CQ - 1),
                            )
                        nc.scalar.activation(
                            qT[:, dc, qf * QB : (qf + 1) * QB],
                            psq[:],
                            AF.Copy,
                            scale=SCALE,
                        )

            # ---- projection units (ctx^T, K^T, V), interleaved later ----
            ctxT_tiles = {}

            def mk_T(jbi):
                def t_unit():
                    b, nkt = jbs[jbi]
                    ct = strm.tile([128, CC, JW], F32R, tag="ctxT", name="ctxT")
                    ctxT_tiles[jbi] = ct
                    for k in range(nkt):
                        t = b + k
                        raw = ld.tile([128, CONTEXT_DIM], F32R, tag="ld", name="ld")
                        nc.gpsimd.dma_start(
                            out=raw[:], in_=ctx_d[t * 128 : (t + 1) * 128, :]
                        )
                        pj = ps_pj.tile([128, 1024], F32R, tag="pj", name="pjt")
                        for c in range(CC):
                            nc.tensor.transpose(
                                pj[:, c * 128 : (c + 1) * 128],
                                raw[:, c * 128 : (c + 1) * 128],
                                ident[:],
                            )
                        nc.vector.tensor_copy(
                            ct[:, :, k * 128 : (k + 1) * 128],
                            pj[:, 0 : CC * 128].rearrange("p (c n) -> p c n", n=128),
                        )

                return t_unit

            def mk_K(jbi, dcp):
                # K^T for j-block jbi, dc pair dcp (dc = 2*dcp, 2*dcp+1)
                def k_unit():
                    b, nkt = jbs[jbi]
                    w = nkt * 128
                    ct = ctxT_tiles[jbi]
                    pj = ps_pj.tile([128, 1024], F32, tag="pj", name="pjk")
                    for dcl in range(2):
                        dc = 2 * dcp + dcl
                        for c in range(CC):
                            nc.tensor.matmul(
                                pj[:, dcl * 512 : dcl * 512 + w],
                                wk[:, c, dc * 128 : (dc + 1) * 128],
                                ct[:, c, 0:w],
                                start=(c == 0),
                                stop=(c == CC - 1),
                            )
                    nc.vector.tensor_copy(
                        kT[:, 2 * dcp : 2 * dcp + 2, b * 128 : b * 128 + w],
                        pj[:].rearrange("p (d x) -> p d x", x=512)[:, :, 0:w],
                    )

                return k_unit

            def mk_V(jbi):
                def v_unit():
                    b, nkt = jbs[jbi]
                    ct = ctxT_tiles[jbi]
                    pj = None
                    for k in range(nkt):
                        t = b + k
                        if k % 2 == 0:
                            pj = ps_pj.tile([128, 1024], F32, tag="pj", name="pjv")
                        sl = pj[:, (k % 2) * 512 : (k % 2) * 512 + 512]
                        for c in range(CC):
                            nc.tensor.matmul(
                                sl,
                                ct[:, c, k * 128 : (k + 1) * 128],
                                wv[:, c, :],
                                start=(c == 0),
                                stop=(c == CC - 1),
                            )
                        v2h = v2[t][:].rearrange("p (h c) -> p h c", c=65)
                        nc.vector.tensor_copy(
                            v2h[:, :, 0:64], sl.rearrange("p (h d) -> p h d", d=64)
                        )
                        nc.gpsimd.tensor_copy(
                            v2h[:, :, 64:65], valid[:, t : t + 1].to_broadcast([128, H, 1])
                        )

                return v_unit

            units = deque()
            for jbi in range(len(jbs)):
                units.append(mk_T(jbi))
                units.append(mk_K(jbi, 0))
                units.append(mk_K(jbi, 1))
                units.append(mk_V(jbi))

            def kt_ready_upto():
                # number of j-blocks fully emitted
                return 0

            # emit all units covering super-block 0 now (prologue)
            n_sb0_jbs = sum(1 for (b, n) in jbs if b < sbs[0][1])
            for _ in range(n_sb0_jbs * 4):
                units.popleft()()

            nc.gpsimd.dma_start(
                out=wo[:], in_=wo_d[:].rearrange("(o p) f -> p o f", p=128)
            )

            # ---- epilogue units ----
            onrm = {}

            def mk_norm(qb):
                def n_unit():
                    q0 = qb * (NQT // NQB)
                    nq = NQT // NQB
                    nc.vector.reciprocal(
                        rcp[:, q0 : q0 + nq, :], O[:, q0 : q0 + nq, :, 64:65]
                    )
                    for qt in range(q0, q0 + nq):
                        st = nrm.tile([128, INNER], F32R, tag="onrm", name="onrm")
                        onrm[qt] = st
                        for h in range(H):
                            nc.vector.tensor_scalar_mul(
                                st[:, h * 64 : (h + 1) * 64],
                                O[:, qt : qt + 1, h : h + 1, 0:64],
                                rcp[:, qt : qt + 1, h : h + 1],
                            )

                return n_unit

            def mk_tr(qt):
                def tr_unit():
                    qb, qtl = qt // (NQT // NQB), qt % (NQT // NQB)
                    pj = ps_pj.tile([128, 1024], F32R, tag="pj", name="pjtr")
                    for c in range(CI):
                        nc.tensor.transpose(
                            pj[:, c * 128 : (c + 1) * 128],
                            onrm[qt][:, c * 128 : (c + 1) * 128],
                            ident[:],
                        )
                    nc.vector.tensor_copy(
                        oT[qb][:, :, qtl * 128 : (qtl + 1) * 128],
                        pj[:, 0 : CI * 128].rearrange("p (c n) -> p c n", n=128),
                    )

                return tr_unit

            def mk_op(qt):
                def op_unit():
                    qb, qtl = qt // (NQT // NQB), qt % (NQT // NQB)
                    pj = ps_pj.tile([128, 1024], F32, tag="pj", name="pjop")
                    for c in range(CI):
                        nc.tensor.matmul(
                            pj[:, 0:512],
                            oT[qb][:, c, qtl * 128 : (qtl + 1) * 128],
                            wo[:, c, :],
                            start=(c == 0),
                            stop=(c == CI - 1),
                        )
                    ot = outp.tile([128, QUERY_DIM], F32, tag="ot", name="ot")
                    nc.vector.tensor_add(ot[:], pj[:, 0:512], bo_bc[:])
                    nc.gpsimd.dma_start(
                        out=out_d[qt * 128 : (qt + 1) * 128, :], in_=ot[:]
                    )

                return op_unit

            # ---- main attention loop ----
            next_unit_jb = n_sb0_jbs  # next jb index not yet emitted

            for sbi, (s0, sn) in enumerate(sbs):
                # make sure units for NEXT super-block get emitted during this one
                groups = [(t0, min(2, s0 + sn - t0)) for t0 in range(s0, s0 + sn, 2)]
                for qb in range(NQB):
                    for hp in range(H // 2):
                        hA, hB = 2 * hp, 2 * hp + 1
                        pvA = ps_pv.tile([128, 512], F32, tag="pv", name="pvA")
                        pvB = ps_pv.tile([128, 512], F32, tag="pv", name="pvB")
                        for t0, tn in groups:
                            if units:
                                units.popleft()()
                            sc = ps_sc.tile([128, 4, 512], F32, tag="sc", name="sc")
                            pt = ptp.tile([128, 4, 512], BF16, tag="pt", name="pt")
                            for j in range(tn):
                                t = t0 + j
                                co = t * 128
                                for hi in range(2):
                                    nc.tensor.matmul(
                                        sc[:, 2 * j + hi, :],
                                        kT[hi * 64 : hi * 64 + 64, hp, co : co + 128],
                                        qT[hi * 64 : hi * 64 + 64, hp, qb * QB : (qb + 1) * QB],
                                        start=True,
                                        stop=True,
                                    )
                            nc.scalar.activation(
                                pt[:, 0 : 2 * tn, :], sc[:, 0 : 2 * tn, :], AF.Exp
                            )
                            for j in range(tn):
                                t = t0 + j
                                for hi, (h, pv) in enumerate(((hA, pvA), (hB, pvB))):
                                    for qc in range(4):
                                        # one accumulation group per PSUM bank:
                                        # start/stop only on the bank's very
                                        # first/last matmul of the super-block
                                        nc.tensor.matmul(
                                            pv[:, qc * 128 : qc * 128 + 65],
                                            pt[:, 2 * j + hi, qc * 128 : (qc + 1) * 128],
                                            v2[t][:, h * 65 : h * 65 + 65],
                                            start=(t == s0 and qc == 0),
                                            stop=(t == s0 + sn - 1 and qc == 3),
                                            skip_group_check=True,
                                        )
                        # drain-add into the SBUF O accumulator
                        for h, pv in ((hA, pvA), (hB, pvB)):
                            src = pv[:].rearrange("p (a x) -> p a x", x=128)[:, :, 0:65]
                            dst = O[:, qb * 4 : qb * 4 + 4, h : h + 1, :]
                            if sbi == 0:
                                nc.vector.tensor_copy(dst, src)
                            else:
                                nc.vector.tensor_add(dst, src, dst)
                        if sbi == len(sbs) - 1 and hp == H // 2 - 1:
                            # all of qb's O rows are final: queue epilogue units
                            units.append(mk_norm(qb))
                            for qt in range(qb * 4, qb * 4 + 4):
                                units.append(mk_tr(qt))
                            for qt in range(qb * 4, qb * 4 + 4):
                                units.append(mk_op(qt))

            while units:
                units.popleft()()

    nc.compile()
    return nc


def kernel(x, context_tensor, mask, Wq, Wk, Wv, Wo, bo):
    from concourse.bass_utils import run_bass_kernel_spmd

    x = np.ascontiguousarray(np.asarray(x, dtype=np.float32))
    context_tensor = np.ascontiguousarray(np.asarray(context_tensor, dtype=np.float32))
    mask = np.asarray(mask)
    Wq = np.ascontiguousarray(np.asarray(Wq, dtype=np.float32))
    Wk = np.ascontiguousarray(np.asarray(Wk, dtype=np.float32))
    Wv = np.ascontiguousarray(np.asarray(Wv, dtype=np.float32))
    Wo = np.ascontiguousarray(np.asarray(Wo, dtype=np.float32))
    bo = np.ascontiguousarray(np.asarray(bo, dtype=np.float32))

    # host-side context compaction using the mask
    meffs = [int(mask[b].sum()) for b in range(B)]
    m_pad = max(M_PAD_MIN, ((max(meffs) + 127) // 128) * 128)
    ctx_c = np.zeros((B, m_pad, CONTEXT_DIM), dtype=np.float32)
    val = np.zeros((B, m_pad), dtype=np.float32)
    for b in range(B):
        idx = np.flatnonzero(mask[b])
        ctx_c[b, : len(idx)] = context_tensor[b, idx]
        val[b, : len(idx)] = 1.0

    if m_pad not in _compiled:
        _compiled[m_pad] = _build(m_pad)
    nc = _compiled[m_pad]

    rows_per_core = N // (NCORES // B)  # 1024
    in_maps = []
    for d in range(NCORES):
        b = d // (NCORES // B)
        r0 = (d % (NCORES // B)) * rows_per_core
        in_maps.append(
            {
                "xs": x[b, r0 : r0 + rows_per_core],
                "ctx": ctx_c[b],
                "valid": val[b],
                "Wq": Wq,
                "Wk": Wk,
                "Wv": Wv,
                "Wo": Wo,
                "bo": bo,
            }
        )

    res = run_bass_kernel_spmd(nc, in_maps, list(range(NCORES)))
    out = np.empty((B, N, QUERY_DIM), dtype=np.float32)
    for d in range(NCORES):
        b = d // (NCORES // B)
        r0 = (d % (NCORES // B)) * rows_per_core
        out[b, r0 : r0 + rows_per_core] = res.results[d]["out"]
    return out


# revision 9
# speedup vs baseline: 1.3470x; 1.3470x over previous
"""Cross-attention Bass kernel for 8 trn2 NeuronCores.

Sharding: core d handles batch b = d//4 and query rows [(d%4)*1024, (d%4+1)*1024)
of that batch, computing all 8 heads (no collectives). The context is compacted
on the host using the mask (masked rows dropped, zero-padded to a multiple of
128), which preserves softmax semantics exactly while nearly halving the
attention work.

Device dataflow (v2 — k-blocked, engine-balanced):
  - Q^T = Wq^T x^T (bf16, softmax scale folded into the drain), K^T = Wk^T ctx^T
    (bf16), V natural = ctx^T-contracted with Wv (bf16) with a per-head "ones"
    column carrying the valid mask.
  - Scores computed transposed per 2-ktile group: S^T[k, q] in a single 4-bank
    PSUM tile (2 ktiles x 2 heads); one big exp on ScalarE -> P^T bf16.
  - PV in NATURAL orientation: O[q, 65] += P^T-chunk.T @ [V|valid] with bf16
    operands (free dim 65 -> half the PE rows of the O^T orientation).
    Accumulation chains span an 8-ktile super-block in PSUM, then drain-add
    into an SBUF O accumulator on VectorE. The softmax denominator rides along
    as column 64, so normalization is a per-partition scalar multiply - no
    DRAM broadcast round-trips.
  - K/V/ctx^T projection work is split into small units and interleaved into
    the attention group loop (one unit per score group), so the Tensor engine
    never idles while ScalarE exps and vice versa.
  - Epilogue: normalize on VectorE, PE-transpose O, output projection, bias on
    VectorE, store via Pool-engine DMA.

Engine budget per core (cost-model rows): PE ~370k cycles, ACT ~135us (exp),
DVE ~80us, DMA ~47us.
"""
import numpy as np

B, N, M = 2, 4096, 4096
QUERY_DIM, CONTEXT_DIM = 512, 768
H, D = 8, 64
INNER = H * D  # 512
NCORES = 8
N_DEV = (B * N) // NCORES  # 1024 query rows per core
M_PAD_MIN = 256

_compiled = {}


def _build(m_pad):
    from collections import deque

    from concourse import bacc
    import concourse.bass as bass
    import concourse.mybir as mybir
    import concourse.tile as tile
    from concourse.masks import make_identity

    F32 = mybir.dt.float32
    F32R = mybir.dt.float32r
    BF16 = mybir.dt.bfloat16
    AF = mybir.ActivationFunctionType

    KT = m_pad // 128
    SCALE = float(D) ** -0.5
    CQ = QUERY_DIM // 128  # 4
    CC = CONTEXT_DIM // 128  # 6
    CI = INNER // 128  # 4
    NQT = N_DEV // 128  # 8
    QB = 512
    NQB = N_DEV // QB  # 2

    SB = 8  # ktiles per PV-accumulation super-block
    sbs = [(s, min(SB, KT - s)) for s in range(0, KT, SB)]
    # projection j-blocks: pairs of ktiles; fold an odd trailing ktile into a
    # final 3-wide block so every K-projection matmul keeps free dim >= 256
    if KT % 2 == 1 and KT >= 3:
        jbs = [(b, 2) for b in range(0, KT - 3, 2)] + [(KT - 3, 3)]
    else:
        jbs = [(b, min(2, KT - b)) for b in range(0, KT, 2)]
    JW = 384 if (KT % 2 == 1 and KT >= 3) else 256  # widest j-block

    nc = bacc.Bacc()
    xs_d = nc.declare_dram_parameter("xs", [N_DEV, QUERY_DIM], F32, isOutput=False)
    ctx_d = nc.declare_dram_parameter("ctx", [m_pad, CONTEXT_DIM], F32, isOutput=False)
    val_d = nc.declare_dram_parameter("valid", [m_pad], F32, isOutput=False)
    wq_d = nc.declare_dram_parameter("Wq", [QUERY_DIM, INNER], F32, isOutput=False)
    wk_d = nc.declare_dram_parameter("Wk", [CONTEXT_DIM, INNER], F32, isOutput=False)
    wv_d = nc.declare_dram_parameter("Wv", [CONTEXT_DIM, INNER], F32, isOutput=False)
    wo_d = nc.declare_dram_parameter("Wo", [INNER, QUERY_DIM], F32, isOutput=False)
    bo_d = nc.declare_dram_parameter("bo", [QUERY_DIM], F32, isOutput=False)
    out_d = nc.declare_dram_parameter("out", [N_DEV, QUERY_DIM], F32, isOutput=True)

    with tile.TileContext(nc) as tc:
        with (
            tc.tile_pool(name="big", bufs=1) as big,
            tc.tile_pool(name="strm", bufs=2) as strm,
            tc.tile_pool(name="ld", bufs=3) as ld,
            tc.tile_pool(name="ptp", bufs=5) as ptp,
            tc.tile_pool(name="outp", bufs=2) as outp,
            tc.tile_pool(name="nrm", bufs=4) as nrm,
            tc.tile_pool(name="ps_scA", bufs=1, space="PSUM") as ps_scA,
            tc.tile_pool(name="ps_scB", bufs=1, space="PSUM") as ps_scB,
            tc.tile_pool(name="ps_pv", bufs=2, space="PSUM") as ps_pv,
            tc.tile_pool(name="ps_pj", bufs=1, space="PSUM") as ps_pj,
        ):
            # ---- persistent SBUF tiles ----
            qT = big.tile([128, CI, N_DEV], BF16, tag="qT", name="qT")
            kT = big.tile([128, CI, m_pad], BF16, tag="kT", name="kT")
            v2 = [
                big.tile([128, H * 65], BF16, tag=f"v2_{t}", name=f"v2_{t}")
                for t in range(KT)
            ]
            # O accumulator: [q-tile, head, 64 dims + denominator]
            O = big.tile([128, NQT, H, 65], F32R, tag="O", name="O")
            rcp = big.tile([128, NQT, H], F32, tag="rcp", name="rcp")
            wo = big.tile([128, CI, QUERY_DIM], F32R, tag="wo", name="wo")
            bo_bc = big.tile([128, QUERY_DIM], F32, tag="bo", name="bo")
            valid = big.tile([128, KT], F32, tag="valid", name="valid")
            oT = [
                big.tile([128, CI, QB], F32R, tag=f"oT{qb}", name=f"oT{qb}")
                for qb in range(NQB)
            ]
            wk = big.tile([128, CC, INNER], F32R, tag="wk", name="wk")
            wv = big.tile([128, CC, INNER], F32R, tag="wv", name="wv")
            identf = big.tile([128, 128], F32, tag="identf", name="identf")
            ident = big.tile([128, 128], F32R, tag="ident", name="ident")

            # ---- prologue: x^T and Q^T ----
            with tc.tile_pool(name="pro", bufs=1) as pro:
                xs = pro.tile([128, NQT, QUERY_DIM], F32R, tag="xs", name="xs")
                xT = pro.tile([128, CQ, N_DEV], F32R, tag="xT", name="xT")
                wq = pro.tile([128, CQ, INNER], F32R, tag="wq", name="wq")
                # DMA priority order on the sync queue
                nc.gpsimd.dma_start(
                    out=xs[:], in_=xs_d[:].rearrange("(t p) f -> p t f", p=128)
                )
                nc.gpsimd.dma_start(
                    out=wq[:], in_=wq_d[:].rearrange("(o p) f -> p o f", p=128)
                )
                nc.gpsimd.dma_start(
                    out=wk[:], in_=wk_d[:].rearrange("(o p) f -> p o f", p=128)
                )
                nc.gpsimd.dma_start(
                    out=wv[:], in_=wv_d[:].rearrange("(o p) f -> p o f", p=128)
                )
                nc.sync.dma_start(
                    out=valid[:], in_=val_d[:].rearrange("(t p) -> p t", p=128)
                )
                nc.sync.dma_start(
                    out=bo_bc[:],
                    in_=bass.AP(tensor=bo_d, offset=0, ap=[[0, 128], [1, QUERY_DIM]]),
                )
                make_identity(nc, identf[:])
                nc.vector.tensor_copy(ident[:], identf[:])

                # x^T: ping-pong transposes across the two psum pools
                for nt in range(NQT):
                    if nt % 2 == 0:
                        pj = ps_pj.tile([128, 1024], F32R, tag="pj", name="pjx")
                        dst = pj
                    else:
                        pvps = ps_pv.tile([128, 512], F32R, tag="pv", name="pvx")
                        dst = pvps
                    for c in range(CQ):
                        nc.tensor.transpose(
                            dst[:, c * 128 : (c + 1) * 128],
                            xs[:, nt, c * 128 : (c + 1) * 128],
                            ident[:],
                        )
                    nc.vector.tensor_copy(
                        xT[:, :, nt * 128 : (nt + 1) * 128],
                        dst[:, 0 : CQ * 128].rearrange("p (c n) -> p c n", n=128),
                    )

                # Q^T (scale folded into the ACT drain, bf16 out)
                for dc in range(CI):
                    for qf in range(N_DEV // QB):
                        psq = ps_pv.tile([128, 512], F32, tag="pv", name="psq")
                        for c in range(CQ):
                            nc.tensor.matmul(
                                psq[:],
                                wq[:, c, dc * 128 : (dc + 1) * 128],
                                xT[:, c, qf * QB : (qf + 1) * QB],
                                start=(c == 0),
                                stop=(c == CQ - 1),
                            )
                        nc.scalar.activation(
                            qT[:, dc, qf * QB : (qf + 1) * QB],
                            psq[:],
                            AF.Copy,
                            scale=SCALE,
                        )

            # ---- projection units (ctx^T, K^T, V), interleaved later ----
            ctxT_tiles = {}

            def mk_T(jbi, k):
                def t_unit():
                    b, nkt = jbs[jbi]
                    if k == 0:
                        ctxT_tiles[jbi] = strm.tile(
                            [128, CC, JW], F32R, tag="ctxT", name="ctxT"
                        )
                    ct = ctxT_tiles[jbi]
                    t = b + k
                    raw = ld.tile([128, CONTEXT_DIM], F32R, tag="ld", name="ld")
                    nc.gpsimd.dma_start(
                        out=raw[:], in_=ctx_d[t * 128 : (t + 1) * 128, :]
                    )
                    pj = ps_pj.tile([128, 1024], F32R, tag="pj", name="pjt")
                    for c in range(CC):
                        nc.tensor.transpose(
                            pj[:, c * 128 : (c + 1) * 128],
                            raw[:, c * 128 : (c + 1) * 128],
                            ident[:],
                        )
                    nc.vector.tensor_copy(
                        ct[:, :, k * 128 : (k + 1) * 128],
                        pj[:, 0 : CC * 128].rearrange("p (c n) -> p c n", n=128),
                    )

                return t_unit

            def mk_K(jbi, dcp):
                # K^T for j-block jbi, dc pair dcp (dc = 2*dcp, 2*dcp+1)
                def k_unit():
                    b, nkt = jbs[jbi]
                    w = nkt * 128
                    ct = ctxT_tiles[jbi]
                    pj = ps_pj.tile([128, 1024], F32, tag="pj", name="pjk")
                    for dcl in range(2):
                        dc = 2 * dcp + dcl
                        for c in range(CC):
                            nc.tensor.matmul(
                                pj[:, dcl * 512 : dcl * 512 + w],
                                wk[:, c, dc * 128 : (dc + 1) * 128],
                                ct[:, c, 0:w],
                                start=(c == 0),
                                stop=(c == CC - 1),
                            )
                    nc.vector.tensor_copy(
                        kT[:, 2 * dcp : 2 * dcp + 2, b * 128 : b * 128 + w],
                        pj[:].rearrange("p (d x) -> p d x", x=512)[:, :, 0:w],
                    )

                return k_unit

            def mk_V(jbi, k):
                def v_unit():
                    b, nkt = jbs[jbi]
                    ct = ctxT_tiles[jbi]
                    t = b + k
                    pj = ps_pj.tile([128, 1024], F32, tag="pj", name="pjv")
                    sl = pj[:, 0:512]
                    for c in range(CC):
                        nc.tensor.matmul(
                            sl,
                            ct[:, c, k * 128 : (k + 1) * 128],
                            wv[:, c, :],
                            start=(c == 0),
                            stop=(c == CC - 1),
                        )
                    v2h = v2[t][:].rearrange("p (h c) -> p h c", c=65)
                    nc.vector.tensor_copy(
                        v2h[:, :, 0:64], sl.rearrange("p (h d) -> p h d", d=64)
                    )
                    nc.gpsimd.tensor_copy(
                        v2h[:, :, 64:65], valid[:, t : t + 1].to_broadcast([128, H, 1])
                    )

                return v_unit

            units = deque()
            for jbi in range(len(jbs)):
                b, nkt = jbs[jbi]
                for k in range(nkt):
                    units.append(mk_T(jbi, k))
                units.append(mk_K(jbi, 0))
                units.append(mk_K(jbi, 1))
                for k in range(nkt):
                    units.append(mk_V(jbi, k))

            # emit all units covering super-block 0 now (prologue)
            n_sb0 = sum(2 * n + 2 for (b, n) in jbs if b < sbs[0][1])
            for _ in range(n_sb0):
                units.popleft()()

            nc.gpsimd.dma_start(
                out=wo[:], in_=wo_d[:].rearrange("(o p) f -> p o f", p=128)
            )

            # ---- epilogue units ----
            onrm = {}

            def mk_norm(qb):
                def n_unit():
                    q0 = qb * (NQT // NQB)
                    nq = NQT // NQB
                    nc.vector.reciprocal(
                        rcp[:, q0 : q0 + nq, :], O[:, q0 : q0 + nq, :, 64:65]
                    )
                    for qt in range(q0, q0 + nq):
                        st = nrm.tile([128, INNER], F32R, tag="onrm", name="onrm")
                        onrm[qt] = st
                        for h in range(H):
                            nc.vector.tensor_scalar_mul(
                                st[:, h * 64 : (h + 1) * 64],
                                O[:, qt : qt + 1, h : h + 1, 0:64],
                                rcp[:, qt : qt + 1, h : h + 1],
                            )

                return n_unit

            def mk_tr(qt):
                def tr_unit():
                    qb, qtl = qt // (NQT // NQB), qt % (NQT // NQB)
                    pj = ps_pj.tile([128, 1024], F32R, tag="pj", name="pjtr")
                    for c in range(CI):
                        nc.tensor.transpose(
                            pj[:, c * 128 : (c + 1) * 128],
                            onrm[qt][:, c * 128 : (c + 1) * 128],
                            ident[:],
                        )
                    nc.vector.tensor_copy(
                        oT[qb][:, :, qtl * 128 : (qtl + 1) * 128],
                        pj[:, 0 : CI * 128].rearrange("p (c n) -> p c n", n=128),
                    )

                return tr_unit

            def mk_op(qt):
                def op_unit():
                    qb, qtl = qt // (NQT // NQB), qt % (NQT // NQB)
                    pj = ps_pj.tile([128, 1024], F32, tag="pj", name="pjop")
                    for c in range(CI):
                        nc.tensor.matmul(
                            pj[:, 0:512],
                            oT[qb][:, c, qtl * 128 : (qtl + 1) * 128],
                            wo[:, c, :],
                            start=(c == 0),
                            stop=(c == CI - 1),
                        )
                    ot = outp.tile([128, QUERY_DIM], F32, tag="ot", name="ot")
                    nc.vector.tensor_add(ot[:], pj[:, 0:512], bo_bc[:])
                    nc.gpsimd.dma_start(
                        out=out_d[qt * 128 : (qt + 1) * 128, :], in_=ot[:]
                    )

                return op_unit

            # ---- main attention loop ----
            for sbi, (s0, sn) in enumerate(sbs):
                # make sure units for NEXT super-block get emitted during this one
                groups = [(t0, min(2, s0 + sn - t0)) for t0 in range(s0, s0 + sn, 2)]
                for qb in range(NQB):
                    for hp in range(H // 2):
                        hA, hB = 2 * hp, 2 * hp + 1
                        pvA = ps_pv.tile([128, 512], F32, tag="pv", name="pvA")
                        pvB = ps_pv.tile([128, 512], F32, tag="pv", name="pvB")
                        for t0, tn in groups:
                            if units:
                                units.popleft()()
                            # scores per head into separate 2-bank tiles so
                            # exp(head A) overlaps the S matmuls of head B
                            pts = []
                            for hi, scp in ((0, ps_scA), (1, ps_scB)):
                                sc = scp.tile(
                                    [128, 2, 512], F32, tag="sc", name="sc"
                                )
                                pt = ptp.tile(
                                    [128, 2, 512], BF16, tag="pt", name="pt"
                                )
                                pts.append(pt)
                                for j in range(tn):
                                    t = t0 + j
                                    co = t * 128
                                    nc.tensor.matmul(
                                        sc[:, j, :],
                                        kT[hi * 64 : hi * 64 + 64, hp, co : co + 128],
                                        qT[hi * 64 : hi * 64 + 64, hp, qb * QB : (qb + 1) * QB],
                                        start=True,
                                        stop=True,
                                    )
                                nc.scalar.activation(
                                    pt[:, 0:tn, :], sc[:, 0:tn, :], AF.Exp
                                )
                            for hi, (h, pv) in enumerate(((hA, pvA), (hB, pvB))):
                                for j in range(tn):
                                    t = t0 + j
                                    for qc in range(4):
                                        # one accumulation group per PSUM bank:
                                        # start/stop only on the bank's very
                                        # first/last matmul of the super-block
                                        nc.tensor.matmul(
                                            pv[:, qc * 128 : qc * 128 + 65],
                                            pts[hi][:, j, qc * 128 : (qc + 1) * 128],
                                            v2[t][:, h * 65 : h * 65 + 65],
                                            start=(t == s0 and qc == 0),
                                            stop=(t == s0 + sn - 1 and qc == 3),
                                            skip_group_check=True,
                                        )
                        # drain-add into the SBUF O accumulator
                        for h, pv in ((hA, pvA), (hB, pvB)):
                            src = pv[:].rearrange("p (a x) -> p a x", x=128)[:, :, 0:65]
                            dst = O[:, qb * 4 : qb * 4 + 4, h : h + 1, :]
                            if sbi == 0:
                                nc.vector.tensor_copy(dst, src)
                            else:
                                nc.vector.tensor_add(dst, src, dst)
                        if sbi == len(sbs) - 1 and hp == H // 2 - 1:
                            # all of qb's O rows are final: queue epilogue units
                            units.append(mk_norm(qb))
                            for qt in range(qb * 4, qb * 4 + 4):
                                units.append(mk_tr(qt))
                            for qt in range(qb * 4, qb * 4 + 4):
                                units.append(mk_op(qt))

            while units:
                units.popleft()()

    nc.compile()
    return nc


def kernel(x, context_tensor, mask, Wq, Wk, Wv, Wo, bo):
    from concourse.bass_utils import run_bass_kernel_spmd

    x = np.ascontiguousarray(np.asarray(x, dtype=np.float32))
    context_tensor = np.ascontiguousarray(np.asarray(context_tensor, dtype=np.float32))
    mask = np.asarray(mask)
    Wq = np.ascontiguousarray(np.asarray(Wq, dtype=np.float32))
    Wk = np.ascontiguousarray(np.asarray(Wk, dtype=np.float32))
    Wv = np.ascontiguousarray(np.asarray(Wv, dtype=np.float32))
    Wo = np.ascontiguousarray(np.asarray(Wo, dtype=np.float32))
    bo = np.ascontiguousarray(np.asarray(bo, dtype=np.float32))

    # host-side context compaction using the mask
    meffs = [int(mask[b].sum()) for b in range(B)]
    m_pad = max(M_PAD_MIN, ((max(meffs) + 127) // 128) * 128)
    ctx_c = np.zeros((B, m_pad, CONTEXT_DIM), dtype=np.float32)
    val = np.zeros((B, m_pad), dtype=np.float32)
    for b in range(B):
        idx = np.flatnonzero(mask[b])
        ctx_c[b, : len(idx)] = context_tensor[b, idx]
        val[b, : len(idx)] = 1.0

    if m_pad not in _compiled:
        _compiled[m_pad] = _build(m_pad)
    nc = _compiled[m_pad]

    rows_per_core = N // (NCORES // B)  # 1024
    in_maps = []
    for d in range(NCORES):
        b = d // (NCORES // B)
        r0 = (d % (NCORES // B)) * rows_per_core
        in_maps.append(
            {
                "xs": x[b, r0 : r0 + rows_per_core],
                "ctx": ctx_c[b],
                "valid": val[b],
                "Wq": Wq,
                "Wk": Wk,
                "Wv": Wv,
                "Wo": Wo,
                "bo": bo,
            }
        )

    res = run_bass_kernel_spmd(nc, in_maps, list(range(NCORES)))
    out = np.empty((B, N, QUERY_DIM), dtype=np.float32)
    for d in range(NCORES):
        b = d // (NCORES // B)
        r0 = (d % (NCORES // B)) * rows_per_core
        out[b, r0 : r0 + rows_per_core] = res.results[d]["out"]
    return out


# revision 10
# speedup vs baseline: 1.5013x; 1.1145x over previous
"""Cross-attention Bass kernel for 8 trn2 NeuronCores.

Sharding: core d handles batch b = d//4 and query rows [(d%4)*1024, (d%4+1)*1024)
of that batch, computing all 8 heads (no collectives). The context is compacted
on the host using the mask (masked rows dropped, zero-padded to a multiple of
128), which preserves softmax semantics exactly while nearly halving the
attention work.

Device dataflow (v2 — k-blocked, engine-balanced):
  - Q^T = Wq^T x^T (bf16, softmax scale folded into the drain), K^T = Wk^T ctx^T
    (bf16), V natural = ctx^T-contracted with Wv (bf16) with a per-head "ones"
    column carrying the valid mask.
  - Scores computed transposed per 2-ktile group: S^T[k, q] in a single 4-bank
    PSUM tile (2 ktiles x 2 heads); one big exp on ScalarE -> P^T bf16.
  - PV in NATURAL orientation: O[q, 65] += P^T-chunk.T @ [V|valid] with bf16
    operands (free dim 65 -> half the PE rows of the O^T orientation).
    Accumulation chains span an 8-ktile super-block in PSUM, then drain-add
    into an SBUF O accumulator on VectorE. The softmax denominator rides along
    as column 64, so normalization is a per-partition scalar multiply - no
    DRAM broadcast round-trips.
  - K/V/ctx^T projection work is split into small units and interleaved into
    the attention group loop (one unit per score group), so the Tensor engine
    never idles while ScalarE exps and vice versa.
  - Epilogue: normalize on VectorE, PE-transpose O, output projection, bias on
    VectorE, store via Pool-engine DMA.

Engine budget per core (cost-model rows): PE ~370k cycles, ACT ~135us (exp),
DVE ~80us, DMA ~47us.
"""
import numpy as np

B, N, M = 2, 4096, 4096
QUERY_DIM, CONTEXT_DIM = 512, 768
H, D = 8, 64
INNER = H * D  # 512
NCORES = 8
N_DEV = (B * N) // NCORES  # 1024 query rows per core
M_PAD_MIN = 256

_compiled = {}


def _build(m_pad):
    from collections import deque

    from concourse import bacc
    import concourse.bass as bass
    import concourse.mybir as mybir
    import concourse.tile as tile
    from concourse.masks import make_identity

    F32 = mybir.dt.float32
    F32R = mybir.dt.float32r
    BF16 = mybir.dt.bfloat16
    AF = mybir.ActivationFunctionType

    KT = m_pad // 128
    SCALE = float(D) ** -0.5
    CQ = QUERY_DIM // 128  # 4
    CC = CONTEXT_DIM // 128  # 6
    CI = INNER // 128  # 4
    NQT = N_DEV // 128  # 8
    QB = 512
    NQB = N_DEV // QB  # 2

    SB = 4  # ktiles per PV-accumulation super-block
    sbs = [(s, min(SB, KT - s)) for s in range(0, KT, SB)]
    # projection j-blocks: pairs of ktiles; fold an odd trailing ktile into a
    # final 3-wide block so every K-projection matmul keeps free dim >= 256
    if KT % 2 == 1 and KT >= 3:
        jbs = [(b, 2) for b in range(0, KT - 3, 2)] + [(KT - 3, 3)]
    else:
        jbs = [(b, min(2, KT - b)) for b in range(0, KT, 2)]
    JW = 384 if (KT % 2 == 1 and KT >= 3) else 256  # widest j-block

    nc = bacc.Bacc()
    xs_d = nc.declare_dram_parameter("xs", [N_DEV, QUERY_DIM], F32, isOutput=False)
    ctx_d = nc.declare_dram_parameter("ctx", [m_pad, CONTEXT_DIM], F32, isOutput=False)
    val_d = nc.declare_dram_parameter("valid", [m_pad], F32, isOutput=False)
    wq_d = nc.declare_dram_parameter("Wq", [QUERY_DIM, INNER], F32, isOutput=False)
    wk_d = nc.declare_dram_parameter("Wk", [CONTEXT_DIM, INNER], F32, isOutput=False)
    wv_d = nc.declare_dram_parameter("Wv", [CONTEXT_DIM, INNER], F32, isOutput=False)
    wo_d = nc.declare_dram_parameter("Wo", [INNER, QUERY_DIM], F32, isOutput=False)
    bo_d = nc.declare_dram_parameter("bo", [QUERY_DIM], F32, isOutput=False)
    out_d = nc.declare_dram_parameter("out", [N_DEV, QUERY_DIM], F32, isOutput=True)

    with tile.TileContext(nc) as tc:
        with (
            tc.tile_pool(name="big", bufs=1) as big,
            tc.tile_pool(name="strm", bufs=2) as strm,
            tc.tile_pool(name="ld", bufs=3) as ld,
            tc.tile_pool(name="ptp", bufs=6) as ptp,
            tc.tile_pool(name="outp", bufs=2) as outp,
            tc.tile_pool(name="nrm", bufs=4) as nrm,
            tc.tile_pool(name="ps_scA", bufs=1, space="PSUM") as ps_scA,
            tc.tile_pool(name="ps_scB", bufs=1, space="PSUM") as ps_scB,
            tc.tile_pool(name="ps_pv", bufs=2, space="PSUM") as ps_pv,
            tc.tile_pool(name="ps_pj", bufs=1, space="PSUM") as ps_pj,
        ):
            # ---- persistent SBUF tiles ----
            qT = big.tile([128, CI, N_DEV], BF16, tag="qT", name="qT")
            kT = big.tile([128, CI, m_pad], BF16, tag="kT", name="kT")
            v2 = [
                big.tile([128, H * 65], BF16, tag=f"v2_{t}", name=f"v2_{t}")
                for t in range(KT)
            ]
            # O accumulator: [q-tile, head, 64 dims + denominator]
            O = big.tile([128, NQT, H, 65], F32R, tag="O", name="O")
            rcp = big.tile([128, NQT, H], F32, tag="rcp", name="rcp")
            wo = big.tile([128, CI, QUERY_DIM], F32R, tag="wo", name="wo")
            bo_bc = big.tile([128, QUERY_DIM], F32, tag="bo", name="bo")
            valid = big.tile([128, KT], F32, tag="valid", name="valid")
            oT = [
                big.tile([128, CI, QB], F32R, tag=f"oT{qb}", name=f"oT{qb}")
                for qb in range(NQB)
            ]
            wk = big.tile([128, CC, INNER], F32R, tag="wk", name="wk")
            wv = big.tile([128, CC, INNER], F32R, tag="wv", name="wv")
            identf = big.tile([128, 128], F32, tag="identf", name="identf")
            ident = big.tile([128, 128], F32R, tag="ident", name="ident")

            # ---- prologue: x^T and Q^T ----
            with tc.tile_pool(name="pro", bufs=1) as pro:
                xs = pro.tile([128, NQT, QUERY_DIM], F32R, tag="xs", name="xs")
                xT = pro.tile([128, CQ, N_DEV], F32R, tag="xT", name="xT")
                wq = pro.tile([128, CQ, INNER], F32R, tag="wq", name="wq")
                # DMA priority order on the sync queue (ctx tiles go on the
                # gpsimd queue concurrently); f32r tiles take a free bitcast
                xs_r = xs_d[:].rearrange("(t p) f -> p t f", p=128).bitcast(F32R)
                nc.sync.dma_start(out=xs[:, 0:4, :], in_=xs_r[:, 0:4, :])
                nc.sync.dma_start(
                    out=wq[:],
                    in_=wq_d[:].rearrange("(o p) f -> p o f", p=128).bitcast(F32R),
                )
                nc.sync.dma_start(out=xs[:, 4:8, :], in_=xs_r[:, 4:8, :])
                nc.sync.dma_start(
                    out=wk[:],
                    in_=wk_d[:].rearrange("(o p) f -> p o f", p=128).bitcast(F32R),
                )
                nc.sync.dma_start(
                    out=wv[:],
                    in_=wv_d[:].rearrange("(o p) f -> p o f", p=128).bitcast(F32R),
                )
                nc.sync.dma_start(
                    out=valid[:], in_=val_d[:].rearrange("(t p) -> p t", p=128)
                )
                nc.sync.dma_start(
                    out=bo_bc[:],
                    in_=bass.AP(tensor=bo_d, offset=0, ap=[[0, 128], [1, QUERY_DIM]]),
                )
                make_identity(nc, identf[:])
                nc.vector.tensor_copy(ident[:], identf[:])

                # x^T then Q^T, half the q-range at a time so Q^T work
                # starts as soon as the first xs DMA lands
                for qf in range(N_DEV // QB):
                    for nt in range(qf * 4, qf * 4 + 4):
                        if nt % 2 == 0:
                            dst = ps_pj.tile([128, 1024], F32R, tag="pj", name="pjx")
                        else:
                            dst = ps_pv.tile([128, 512], F32R, tag="pv", name="pvx")
                        for c in range(CQ):
                            nc.tensor.transpose(
                                dst[:, c * 128 : (c + 1) * 128],
                                xs[:, nt, c * 128 : (c + 1) * 128],
                                ident[:],
                            )
                        nc.vector.tensor_copy(
                            xT[:, :, nt * 128 : (nt + 1) * 128],
                            dst[:, 0 : CQ * 128].rearrange("p (c n) -> p c n", n=128),
                        )
                    for dc in range(CI):
                        psq = ps_pv.tile([128, 512], F32, tag="pv", name="psq")
                        for c in range(CQ):
                            nc.tensor.matmul(
                                psq[:],
                                wq[:, c, dc * 128 : (dc + 1) * 128],
                                xT[:, c, qf * QB : (qf + 1) * QB],
                                start=(c == 0),
                                stop=(c == CQ - 1),
                            )
                        nc.scalar.activation(
                            qT[:, dc, qf * QB : (qf + 1) * QB],
                            psq[:],
                            AF.Copy,
                            scale=SCALE,
                        )

            # ---- projection units (ctx^T, K^T, V), interleaved later ----
            ctxT_tiles = {}

            def mk_T(jbi, k):
                def t_unit():
                    b, nkt = jbs[jbi]
                    if k == 0:
                        ctxT_tiles[jbi] = strm.tile(
                            [128, CC, JW], F32R, tag="ctxT", name="ctxT"
                        )
                    ct = ctxT_tiles[jbi]
                    t = b + k
                    raw = ld.tile([128, CONTEXT_DIM], F32R, tag="ld", name="ld")
                    nc.sync.dma_start(
                        out=raw[:],
                        in_=ctx_d[t * 128 : (t + 1) * 128, :].bitcast(F32R),
                    )
                    pj = ps_pj.tile([128, 1024], F32R, tag="pj", name="pjt")
                    for c in range(CC):
                        nc.tensor.transpose(
                            pj[:, c * 128 : (c + 1) * 128],
                            raw[:, c * 128 : (c + 1) * 128],
                            ident[:],
                        )
                    nc.vector.tensor_copy(
                        ct[:, :, k * 128 : (k + 1) * 128],
                        pj[:, 0 : CC * 128].rearrange("p (c n) -> p c n", n=128),
                    )

                return t_unit

            def mk_K(jbi, dcp):
                # K^T for j-block jbi, dc pair dcp (dc = 2*dcp, 2*dcp+1)
                def k_unit():
                    b, nkt = jbs[jbi]
                    w = nkt * 128
                    ct = ctxT_tiles[jbi]
                    pj = ps_pj.tile([128, 1024], F32, tag="pj", name="pjk")
                    for dcl in range(2):
                        dc = 2 * dcp + dcl
                        for c in range(CC):
                            nc.tensor.matmul(
                                pj[:, dcl * 512 : dcl * 512 + w],
                                wk[:, c, dc * 128 : (dc + 1) * 128],
                                ct[:, c, 0:w],
                                start=(c == 0),
                                stop=(c == CC - 1),
                            )
                    nc.vector.tensor_copy(
                        kT[:, 2 * dcp : 2 * dcp + 2, b * 128 : b * 128 + w],
                        pj[:].rearrange("p (d x) -> p d x", x=512)[:, :, 0:w],
                    )

                return k_unit

            def mk_V(jbi, k):
                def v_unit():
                    b, nkt = jbs[jbi]
                    ct = ctxT_tiles[jbi]
                    t = b + k
                    pj = ps_pj.tile([128, 1024], F32, tag="pj", name="pjv")
                    sl = pj[:, 0:512]
                    for c in range(CC):
                        nc.tensor.matmul(
                            sl,
                            ct[:, c, k * 128 : (k + 1) * 128],
                            wv[:, c, :],
                            start=(c == 0),
                            stop=(c == CC - 1),
                        )
                    v2h = v2[t][:].rearrange("p (h c) -> p h c", c=65)
                    nc.vector.tensor_copy(
                        v2h[:, :, 0:64], sl.rearrange("p (h d) -> p h d", d=64)
                    )
                    nc.gpsimd.tensor_copy(
                        v2h[:, :, 64:65], valid[:, t : t + 1].to_broadcast([128, H, 1])
                    )

                return v_unit

            units = deque()
            for jbi in range(len(jbs)):
                b, nkt = jbs[jbi]
                for k in range(nkt):
                    units.append(mk_T(jbi, k))
                units.append(mk_K(jbi, 0))
                units.append(mk_K(jbi, 1))
                for k in range(nkt):
                    units.append(mk_V(jbi, k))

            # emit all units covering super-block 0 now (prologue)
            n_sb0 = sum(2 * n + 2 for (b, n) in jbs if b < sbs[0][1])
            for _ in range(n_sb0):
                units.popleft()()

            nc.sync.dma_start(
                out=wo[:],
                in_=wo_d[:].rearrange("(o p) f -> p o f", p=128).bitcast(F32R),
            )

            # ---- epilogue units ----
            onrm = {}

            def mk_norm(qb):
                def n_unit():
                    q0 = qb * (NQT // NQB)
                    nq = NQT // NQB
                    nc.vector.reciprocal(
                        rcp[:, q0 : q0 + nq, :], O[:, q0 : q0 + nq, :, 64:65]
                    )
                    for qt in range(q0, q0 + nq):
                        st = nrm.tile([128, INNER], F32R, tag="onrm", name="onrm")
                        onrm[qt] = st
                        for h in range(H):
                            nc.vector.tensor_scalar_mul(
                                st[:, h * 64 : (h + 1) * 64],
                                O[:, qt : qt + 1, h : h + 1, 0:64],
                                rcp[:, qt : qt + 1, h : h + 1],
                            )

                return n_unit

            def mk_tr(qt):
                def tr_unit():
                    qb, qtl = qt // (NQT // NQB), qt % (NQT // NQB)
                    pj = ps_pj.tile([128, 1024], F32R, tag="pj", name="pjtr")
                    for c in range(CI):
                        nc.tensor.transpose(
                            pj[:, c * 128 : (c + 1) * 128],
                            onrm[qt][:, c * 128 : (c + 1) * 128],
                            ident[:],
                        )
                    nc.vector.tensor_copy(
                        oT[qb][:, :, qtl * 128 : (qtl + 1) * 128],
                        pj[:, 0 : CI * 128].rearrange("p (c n) -> p c n", n=128),
                    )

                return tr_unit

            def mk_op(qt):
                def op_unit():
                    qb, qtl = qt // (NQT // NQB), qt % (NQT // NQB)
                    pj = ps_pj.tile([128, 1024], F32, tag="pj", name="pjop")
                    for c in range(CI):
                        nc.tensor.matmul(
                            pj[:, 0:512],
                            oT[qb][:, c, qtl * 128 : (qtl + 1) * 128],
                            wo[:, c, :],
                            start=(c == 0),
                            stop=(c == CI - 1),
                        )
                    ot = outp.tile([128, QUERY_DIM], F32, tag="ot", name="ot")
                    nc.vector.tensor_add(ot[:], pj[:, 0:512], bo_bc[:])
                    nc.gpsimd.dma_start(
                        out=out_d[qt * 128 : (qt + 1) * 128, :], in_=ot[:]
                    )

                return op_unit

            # ---- main attention loop ----
            for sbi, (s0, sn) in enumerate(sbs):
                # make sure units for NEXT super-block get emitted during this one
                groups = [(t0, min(2, s0 + sn - t0)) for t0 in range(s0, s0 + sn, 2)]
                for qb in range(NQB):
                    for hp in range(H // 2):
                        hA, hB = 2 * hp, 2 * hp + 1
                        pvA = ps_pv.tile([128, 512], F32, tag="pv", name="pvA")
                        pvB = ps_pv.tile([128, 512], F32, tag="pv", name="pvB")
                        def emit_pv(t0, tn, pts):
                            for hi, (h, pv) in enumerate(((hA, pvA), (hB, pvB))):
                                for j in range(tn):
                                    t = t0 + j
                                    for qc in range(4):
                                        # one accumulation group per PSUM bank:
                                        # start/stop only on the bank's very
                                        # first/last matmul of the super-block
                                        nc.tensor.matmul(
                                            pv[:, qc * 128 : qc * 128 + 65],
                                            pts[hi][:, j, qc * 128 : (qc + 1) * 128],
                                            v2[t][:, h * 65 : h * 65 + 65],
                                            start=(t == s0 and qc == 0),
                                            stop=(t == s0 + sn - 1 and qc == 3),
                                            skip_group_check=True,
                                        )

                        prev = None
                        for t0, tn in groups:
                            if units:
                                units.popleft()()
                            # scores per head into separate 2-bank tiles so
                            # exp(head A) overlaps the S matmuls of head B;
                            # PV of the previous group runs while exp is in
                            # flight (one-group software pipeline)
                            pts = []
                            for hi, scp in ((0, ps_scA), (1, ps_scB)):
                                sc = scp.tile(
                                    [128, 2, 512], F32, tag="sc", name="sc"
                                )
                                pt = ptp.tile(
                                    [128, 2, 512], BF16, tag="pt", name="pt"
                                )
                                pts.append(pt)
                                for j in range(tn):
                                    t = t0 + j
                                    co = t * 128
                                    nc.tensor.matmul(
                                        sc[:, j, :],
                                        kT[hi * 64 : hi * 64 + 64, hp, co : co + 128],
                                        qT[hi * 64 : hi * 64 + 64, hp, qb * QB : (qb + 1) * QB],
                                        start=True,
                                        stop=True,
                                    )
                                nc.scalar.activation(
                                    pt[:, 0:tn, :], sc[:, 0:tn, :], AF.Exp
                                )
                            if prev is not None:
                                emit_pv(*prev)
                            prev = (t0, tn, pts)
                        emit_pv(*prev)
                        # drain-add into the SBUF O accumulator
                        for h, pv in ((hA, pvA), (hB, pvB)):
                            src = pv[:].rearrange("p (a x) -> p a x", x=128)[:, :, 0:65]
                            dst = O[:, qb * 4 : qb * 4 + 4, h : h + 1, :]
                            if sbi == 0:
                                nc.vector.tensor_copy(dst, src)
                            else:
                                nc.vector.tensor_add(dst, src, dst)
                        if sbi == len(sbs) - 1 and hp == H // 2 - 1:
                            # all of qb's O rows are final: queue epilogue units
                            units.append(mk_norm(qb))
                            for qt in range(qb * 4, qb * 4 + 4):
                                units.append(mk_tr(qt))
                            for qt in range(qb * 4, qb * 4 + 4):
                                units.append(mk_op(qt))

            while units:
                units.popleft()()

    nc.compile()
    return nc


def kernel(x, context_tensor, mask, Wq, Wk, Wv, Wo, bo):
    from concourse.bass_utils import run_bass_kernel_spmd

    x = np.ascontiguousarray(np.asarray(x, dtype=np.float32))
    context_tensor = np.ascontiguousarray(np.asarray(context_tensor, dtype=np.float32))
    mask = np.asarray(mask)
    Wq = np.ascontiguousarray(np.asarray(Wq, dtype=np.float32))
    Wk = np.ascontiguousarray(np.asarray(Wk, dtype=np.float32))
    Wv = np.ascontiguousarray(np.asarray(Wv, dtype=np.float32))
    Wo = np.ascontiguousarray(np.asarray(Wo, dtype=np.float32))
    bo = np.ascontiguousarray(np.asarray(bo, dtype=np.float32))

    # host-side context compaction using the mask
    meffs = [int(mask[b].sum()) for b in range(B)]
    m_pad = max(M_PAD_MIN, ((max(meffs) + 127) // 128) * 128)
    ctx_c = np.zeros((B, m_pad, CONTEXT_DIM), dtype=np.float32)
    val = np.zeros((B, m_pad), dtype=np.float32)
    for b in range(B):
        idx = np.flatnonzero(mask[b])
        ctx_c[b, : len(idx)] = context_tensor[b, idx]
        val[b, : len(idx)] = 1.0

    if m_pad not in _compiled:
        _compiled[m_pad] = _build(m_pad)
    nc = _compiled[m_pad]

    rows_per_core = N // (NCORES // B)  # 1024
    in_maps = []
    for d in range(NCORES):
        b = d // (NCORES // B)
        r0 = (d % (NCORES // B)) * rows_per_core
        in_maps.append(
            {
                "xs": x[b, r0 : r0 + rows_per_core],
                "ctx": ctx_c[b],
                "valid": val[b],
                "Wq": Wq,
                "Wk": Wk,
                "Wv": Wv,
                "Wo": Wo,
                "bo": bo,
            }
        )

    res = run_bass_kernel_spmd(nc, in_maps, list(range(NCORES)))
    out = np.empty((B, N, QUERY_DIM), dtype=np.float32)
    for d in range(NCORES):
        b = d // (NCORES // B)
        r0 = (d % (NCORES // B)) * rows_per_core
        out[b, r0 : r0 + rows_per_core] = res.results[d]["out"]
    return out


# revision 11
# speedup vs baseline: 1.5640x; 1.0418x over previous
"""Cross-attention Bass kernel for 8 trn2 NeuronCores.

Sharding: core d handles batch b = d//4 and query rows [(d%4)*1024, (d%4+1)*1024)
of that batch, computing all 8 heads (no collectives). The context is compacted
on the host using the mask (masked rows dropped, zero-padded to a multiple of
128), which preserves softmax semantics exactly while nearly halving the
attention work.

Device dataflow (k-blocked, engine-balanced, software-pipelined):
  - Q^T = Wq^T x^T (bf16, softmax scale folded into the drain), K^T = Wk^T ctx^T
    (bf16), V natural = ctx^T-contracted with Wv (bf16) with a per-head "ones"
    column carrying the valid mask.
  - Scores per 2-ktile group, split by head into two 2-bank PSUM tiles so the
    exp of head A overlaps the score matmuls of head B (effective double
    buffering inside the 8-bank budget); one exp per head-group on ScalarE
    -> P^T bf16.
  - PV in NATURAL orientation: O[q, 65] += P^T-chunk.T @ [V|valid] with bf16
    operands (free dim 65 -> half the PE rows of the O^T orientation). Each
    PSUM bank holds 4 q-chunk chains as ONE accumulation group (start on the
    bank's first matmul, stop on its last). Chains span a 2-4 ktile
    super-block, then drain-add into an SBUF O accumulator on VectorE. The
    softmax denominator rides along as column 64, so normalization is a
    per-partition scalar multiply - no DRAM broadcast round-trips.
  - PV and drain-adds are emitted one group late (software pipeline), hiding
    the exp latency; K/V/ctx^T projection work is split into small units and
    deadline-paced into the attention group loop so the Tensor engine stays
    busy while ScalarE exps.
  - Epilogue: normalize on VectorE+ScalarE into packed tiles, PE-transpose,
    output projection, bias on VectorE, store via Pool-engine DMA.
"""
import numpy as np

B, N, M = 2, 4096, 4096
QUERY_DIM, CONTEXT_DIM = 512, 768
H, D = 8, 64
INNER = H * D  # 512
NCORES = 8
N_DEV = (B * N) // NCORES  # 1024 query rows per core
M_PAD_MIN = 256

_compiled = {}


def _build(m_pad):
    from collections import deque

    from concourse import bacc
    import concourse.bass as bass
    import concourse.mybir as mybir
    import concourse.tile as tile
    from concourse.masks import make_identity

    F32 = mybir.dt.float32
    F32R = mybir.dt.float32r
    BF16 = mybir.dt.bfloat16
    AF = mybir.ActivationFunctionType

    KT = m_pad // 128
    SCALE = float(D) ** -0.5
    CQ = QUERY_DIM // 128  # 4
    CC = CONTEXT_DIM // 128  # 6
    CI = INNER // 128  # 4
    NQT = N_DEV // 128  # 8
    QB = 512
    NQB = N_DEV // QB  # 2

    # super-blocks (PV accumulation chain extents): small first block so
    # attention starts early, then 4-ktile blocks
    sbs = []
    s = 0
    while s < KT:
        n = min(2 if s == 0 else 4, KT - s)
        sbs.append((s, n))
        s += n
    # projection j-blocks: pairs of ktiles; fold an odd trailing ktile into a
    # final 3-wide block so every K-projection matmul keeps free dim >= 256
    if KT % 2 == 1 and KT >= 3:
        jbs = [(b, 2) for b in range(0, KT - 3, 2)] + [(KT - 3, 3)]
    else:
        jbs = [(b, min(2, KT - b)) for b in range(0, KT, 2)]
    JW = 384 if (KT % 2 == 1 and KT >= 3) else 256  # widest j-block

    nc = bacc.Bacc()
    xs_d = nc.declare_dram_parameter("xs", [N_DEV, QUERY_DIM], F32, isOutput=False)
    ctx_d = nc.declare_dram_parameter("ctx", [m_pad, CONTEXT_DIM], F32, isOutput=False)
    val_d = nc.declare_dram_parameter("valid", [m_pad], F32, isOutput=False)
    wq_d = nc.declare_dram_parameter("Wq", [QUERY_DIM, INNER], F32, isOutput=False)
    wk_d = nc.declare_dram_parameter("Wk", [CONTEXT_DIM, INNER], F32, isOutput=False)
    wv_d = nc.declare_dram_parameter("Wv", [CONTEXT_DIM, INNER], F32, isOutput=False)
    wo_d = nc.declare_dram_parameter("Wo", [INNER, QUERY_DIM], F32, isOutput=False)
    bo_d = nc.declare_dram_parameter("bo", [QUERY_DIM], F32, isOutput=False)
    out_d = nc.declare_dram_parameter("out", [N_DEV, QUERY_DIM], F32, isOutput=True)

    with tile.TileContext(nc) as tc:
        with (
            tc.tile_pool(name="big", bufs=1) as big,
            tc.tile_pool(name="strm", bufs=2) as strm,
            tc.tile_pool(name="ld", bufs=3) as ld,
            tc.tile_pool(name="ptp", bufs=6) as ptp,
            tc.tile_pool(name="outp", bufs=2) as outp,
            tc.tile_pool(name="nrm", bufs=4) as nrm,
            tc.tile_pool(name="ps_scA", bufs=1, space="PSUM") as ps_scA,
            tc.tile_pool(name="ps_scB", bufs=1, space="PSUM") as ps_scB,
            tc.tile_pool(name="ps_pv", bufs=2, space="PSUM") as ps_pv,
            tc.tile_pool(name="ps_pj", bufs=1, space="PSUM") as ps_pj,
        ):
            # ---- persistent SBUF tiles ----
            qT = big.tile([128, CI, N_DEV], BF16, tag="qT", name="qT")
            kT = big.tile([128, CI, m_pad], BF16, tag="kT", name="kT")
            v2 = [
                big.tile([128, H * 65], BF16, tag=f"v2_{t}", name=f"v2_{t}")
                for t in range(KT)
            ]
            # O accumulator: [q-tile, head, 64 dims + denominator]
            O = big.tile([128, NQT, H, 65], F32R, tag="O", name="O")
            rcp = big.tile([128, NQT, H], F32, tag="rcp", name="rcp")
            wo = big.tile([128, CI, QUERY_DIM], F32R, tag="wo", name="wo")
            bo_bc = big.tile([128, QUERY_DIM], F32, tag="bo", name="bo")
            valid = big.tile([128, KT], F32, tag="valid", name="valid")
            oT = [
                big.tile([128, CI, QB], F32R, tag=f"oT{qb}", name=f"oT{qb}")
                for qb in range(NQB)
            ]
            wk = big.tile([128, CC, INNER], F32R, tag="wk", name="wk")
            wv = big.tile([128, CC, INNER], F32R, tag="wv", name="wv")
            identf = big.tile([128, 128], F32, tag="identf", name="identf")
            ident = big.tile([128, 128], F32R, tag="ident", name="ident")

            # ---- prologue: x^T and Q^T ----
            with tc.tile_pool(name="pro", bufs=1) as pro:
                xs = pro.tile([128, NQT, QUERY_DIM], F32R, tag="xs", name="xs")
                xT = pro.tile([128, CQ, N_DEV], F32R, tag="xT", name="xT")
                wq = pro.tile([128, CQ, INNER], F32R, tag="wq", name="wq")
                # big loads on the sync queue (f32r tiles take a free bitcast);
                # wk/wv on the gpsimd queue run concurrently
                xs_r = xs_d[:].rearrange("(t p) f -> p t f", p=128).bitcast(F32R)
                nc.sync.dma_start(out=xs[:, 0:4, :], in_=xs_r[:, 0:4, :])
                nc.sync.dma_start(
                    out=wq[:],
                    in_=wq_d[:].rearrange("(o p) f -> p o f", p=128).bitcast(F32R),
                )
                nc.sync.dma_start(out=xs[:, 4:8, :], in_=xs_r[:, 4:8, :])
                nc.gpsimd.dma_start(
                    out=wk[:], in_=wk_d[:].rearrange("(o p) f -> p o f", p=128)
                )
                nc.gpsimd.dma_start(
                    out=wv[:], in_=wv_d[:].rearrange("(o p) f -> p o f", p=128)
                )
                nc.sync.dma_start(
                    out=valid[:], in_=val_d[:].rearrange("(t p) -> p t", p=128)
                )
                nc.sync.dma_start(
                    out=bo_bc[:],
                    in_=bass.AP(tensor=bo_d, offset=0, ap=[[0, 128], [1, QUERY_DIM]]),
                )
                make_identity(nc, identf[:])
                nc.vector.tensor_copy(ident[:], identf[:])

                # x^T then Q^T, half the q-range at a time so Q^T work starts
                # as soon as the first xs DMA lands
                for qf in range(N_DEV // QB):
                    for nt in range(qf * 4, qf * 4 + 4):
                        if nt % 2 == 0:
                            dst = ps_pj.tile([128, 1024], F32R, tag="pj", name="pjx")
                        else:
                            dst = ps_pv.tile([128, 512], F32R, tag="pv", name="pvx")
                        for c in range(CQ):
                            nc.tensor.transpose(
                                dst[:, c * 128 : (c + 1) * 128],
                                xs[:, nt, c * 128 : (c + 1) * 128],
                                ident[:],
                            )
                        nc.vector.tensor_copy(
                            xT[:, :, nt * 128 : (nt + 1) * 128],
                            dst[:, 0 : CQ * 128].rearrange("p (c n) -> p c n", n=128),
                        )
                    for dc in range(CI):
                        psq = ps_pv.tile([128, 512], F32, tag="pv", name="psq")
                        for c in range(CQ):
                            nc.tensor.matmul(
                                psq[:],
                                wq[:, c, dc * 128 : (dc + 1) * 128],
                                xT[:, c, qf * QB : (qf + 1) * QB],
                                start=(c == 0),
                                stop=(c == CQ - 1),
                            )
                        nc.scalar.activation(
                            qT[:, dc, qf * QB : (qf + 1) * QB],
                            psq[:],
                            AF.Copy,
                            scale=SCALE,
                        )

            # ---- projection units (ctx^T, K^T, V), deadline-paced below ----
            ctxT_tiles = {}

            def mk_T(jbi, k):
                def t_unit():
                    b, nkt = jbs[jbi]
                    if k == 0:
                        ctxT_tiles[jbi] = strm.tile(
                            [128, CC, JW], F32R, tag="ctxT", name="ctxT"
                        )
                    ct = ctxT_tiles[jbi]
                    t = b + k
                    raw = ld.tile([128, CONTEXT_DIM], F32R, tag="ld", name="ld")
                    nc.sync.dma_start(
                        out=raw[:],
                        in_=ctx_d[t * 128 : (t + 1) * 128, :].bitcast(F32R),
                    )
                    pj = ps_pj.tile([128, 1024], F32R, tag="pj", name="pjt")
                    for c in range(CC):
                        nc.tensor.transpose(
                            pj[:, c * 128 : (c + 1) * 128],
                            raw[:, c * 128 : (c + 1) * 128],
                            ident[:],
                        )
                    nc.vector.tensor_copy(
                        ct[:, :, k * 128 : (k + 1) * 128],
                        pj[:, 0 : CC * 128].rearrange("p (c n) -> p c n", n=128),
                    )

                return t_unit

            def mk_K(jbi, dcp):
                # K^T for j-block jbi, dc pair dcp (dc = 2*dcp, 2*dcp+1)
                def k_unit():
                    b, nkt = jbs[jbi]
                    w = nkt * 128
                    ct = ctxT_tiles[jbi]
                    pj = ps_pj.tile([128, 1024], F32, tag="pj", name="pjk")
                    for dcl in range(2):
                        dc = 2 * dcp + dcl
                        for c in range(CC):
                            nc.tensor.matmul(
                                pj[:, dcl * 512 : dcl * 512 + w],
                                wk[:, c, dc * 128 : (dc + 1) * 128],
                                ct[:, c, 0:w],
                                start=(c == 0),
                                stop=(c == CC - 1),
                            )
                    nc.vector.tensor_copy(
                        kT[:, 2 * dcp : 2 * dcp + 2, b * 128 : b * 128 + w],
                        pj[:].rearrange("p (d x) -> p d x", x=512)[:, :, 0:w],
                    )

                return k_unit

            def mk_V(jbi, k):
                def v_unit():
                    b, nkt = jbs[jbi]
                    ct = ctxT_tiles[jbi]
                    t = b + k
                    pj = ps_pj.tile([128, 1024], F32, tag="pj", name="pjv")
                    sl = pj[:, 0:512]
                    for c in range(CC):
                        nc.tensor.matmul(
                            sl,
                            ct[:, c, k * 128 : (k + 1) * 128],
                            wv[:, c, :],
                            start=(c == 0),
                            stop=(c == CC - 1),
                        )
                    v2h = v2[t][:].rearrange("p (h c) -> p h c", c=65)
                    nc.vector.tensor_copy(
                        v2h[:, :, 0:64], sl.rearrange("p (h d) -> p h d", d=64)
                    )
                    nc.gpsimd.tensor_copy(
                        v2h[:, :, 64:65], valid[:, t : t + 1].to_broadcast([128, H, 1])
                    )

                return v_unit

            # (unit_fn, kt_covered_after) - coverage advances when the last
            # unit of a j-block has been emitted
            units = deque()
            for jbi, (b, nkt) in enumerate(jbs):
                seq = (
                    [mk_T(jbi, k) for k in range(nkt)]
                    + [mk_K(jbi, 0), mk_K(jbi, 1)]
                    + [mk_V(jbi, k) for k in range(nkt)]
                )
                for i, fn in enumerate(seq):
                    units.append((fn, b + nkt if i == len(seq) - 1 else 0))

            coverage = [0]

            def pop_unit():
                fn, cov = units.popleft()
                fn()
                if cov:
                    coverage[0] = cov

            # prologue: emit units covering super-block 0
            while units and coverage[0] < sbs[0][1]:
                pop_unit()

            nc.sync.dma_start(
                out=wo[:],
                in_=wo_d[:].rearrange("(o p) f -> p o f", p=128).bitcast(F32R),
            )

            # ---- epilogue units ----
            onrm = {}
            epi = deque()

            def mk_norm(qb):
                def n_unit():
                    q0 = qb * (NQT // NQB)
                    nq = NQT // NQB
                    nc.vector.reciprocal(
                        rcp[:, q0 : q0 + nq, :], O[:, q0 : q0 + nq, :, 64:65]
                    )
                    for qt in range(q0, q0 + nq):
                        st = nrm.tile([128, INNER], F32R, tag="onrm", name="onrm")
                        onrm[qt] = st
                        for h in range(H):
                            if qt % 2 == 0:
                                nc.vector.tensor_scalar_mul(
                                    st[:, h * 64 : (h + 1) * 64],
                                    O[:, qt : qt + 1, h : h + 1, 0:64],
                                    rcp[:, qt : qt + 1, h : h + 1],
                                )
                            else:
                                nc.scalar.activation(
                                    st[:, h * 64 : (h + 1) * 64],
                                    O[:, qt : qt + 1, h : h + 1, 0:64],
                                    AF.Copy,
                                    scale=rcp[:, qt : qt + 1, h : h + 1],
                                )

                return n_unit

            def mk_tr(qt):
                def tr_unit():
                    qb, qtl = qt // (NQT // NQB), qt % (NQT // NQB)
                    pj = ps_pj.tile([128, 1024], F32R, tag="pj", name="pjtr")
                    for c in range(CI):
                        nc.tensor.transpose(
                            pj[:, c * 128 : (c + 1) * 128],
                            onrm[qt][:, c * 128 : (c + 1) * 128],
                            ident[:],
                        )
                    nc.vector.tensor_copy(
                        oT[qb][:, :, qtl * 128 : (qtl + 1) * 128],
                        pj[:, 0 : CI * 128].rearrange("p (c n) -> p c n", n=128),
                    )

                return tr_unit

            def mk_op(qt):
                def op_unit():
                    qb, qtl = qt // (NQT // NQB), qt % (NQT // NQB)
                    pj = ps_pj.tile([128, 1024], F32, tag="pj", name="pjop")
                    for c in range(CI):
                        nc.tensor.matmul(
                            pj[:, 0:512],
                            oT[qb][:, c, qtl * 128 : (qtl + 1) * 128],
                            wo[:, c, :],
                            start=(c == 0),
                            stop=(c == CI - 1),
                        )
                    ot = outp.tile([128, QUERY_DIM], F32, tag="ot", name="ot")
                    nc.vector.tensor_add(ot[:], pj[:, 0:512], bo_bc[:])
                    nc.gpsimd.dma_start(
                        out=out_d[qt * 128 : (qt + 1) * 128, :], in_=ot[:]
                    )

                return op_unit

            # ---- main attention loop (software-pipelined across groups,
            # iterations, and super-blocks) ----
            sched = []
            for sbi, (s0, sn) in enumerate(sbs):
                groups = [(t0, min(2, s0 + sn - t0)) for t0 in range(s0, s0 + sn, 2)]
                for qb in range(NQB):
                    for hp in range(H // 2):
                        sched.append((sbi, s0, sn, qb, hp, groups))

            def emit_pv(t0, tn, pts, pvs, heads, s0, send):
                for hi, (h, pv) in enumerate(zip(heads, pvs)):
                    for j in range(tn):
                        t = t0 + j
                        for qc in range(4):
                            # one accumulation group per PSUM bank: start/stop
                            # only on the bank's first/last matmul of the sb
                            nc.tensor.matmul(
                                pv[:, qc * 128 : qc * 128 + 65],
                                pts[hi][:, j, qc * 128 : (qc + 1) * 128],
                                v2[t][:, h * 65 : h * 65 + 65],
                                start=(t == s0 and qc == 0),
                                stop=(t == send and qc == 3),
                                skip_group_check=True,
                            )

            def emit_drain(pvs, heads, qb, sbi):
                for h, pv in zip(heads, pvs):
                    src = pv[:].rearrange("p (a x) -> p a x", x=128)[:, :, 0:65]
                    dst = O[:, qb * 4 : qb * 4 + 4, h : h + 1, :]
                    if sbi == 0:
                        nc.vector.tensor_copy(dst, src)
                    else:
                        nc.vector.tensor_add(dst, src, dst)
                if sbi == len(sbs) - 1 and heads[0] == H - 2:
                    # all of qb's O rows are final: queue epilogue units
                    epi.append(mk_norm(qb))
                    for qt in range(qb * 4, qb * 4 + 4):
                        epi.append(mk_tr(qt))
                    for qt in range(qb * 4, qb * 4 + 4):
                        epi.append(mk_op(qt))

            prev_pv = None  # (t0, tn, pts, pvs, heads, s0, send)
            pend_drain = None  # (pvs, heads, qb, sbi)

            for sbi, s0, sn, qb, hp, groups in sched:
                hA, hB = 2 * hp, 2 * hp + 1
                pvA = ps_pv.tile([128, 512], F32, tag="pv", name="pvA")
                pvB = ps_pv.tile([128, 512], F32, tag="pv", name="pvB")
                for t0, tn in groups:
                    # deadline-paced projection units: stay ~one super-block
                    # ahead of attention; otherwise drain epilogue units
                    target = min(KT, s0 + sn + 4)
                    pops = 0
                    while units and pops < 2 and coverage[0] < target:
                        pop_unit()
                        pops += 1
                    if pops == 0 and epi:
                        epi.popleft()()
                    # scores per head into separate 2-bank tiles so exp(A)
                    # overlaps the S matmuls of head B
                    pts = []
                    for hi, scp in ((0, ps_scA), (1, ps_scB)):
                        sc = scp.tile([128, 2, 512], F32, tag="sc", name="sc")
                        pt = ptp.tile([128, 2, 512], BF16, tag="pt", name="pt")
                        pts.append(pt)
                        for j in range(tn):
                            t = t0 + j
                            co = t * 128
                            nc.tensor.matmul(
                                sc[:, j, :],
                                kT[hi * 64 : hi * 64 + 64, hp, co : co + 128],
                                qT[hi * 64 : hi * 64 + 64, hp, qb * QB : (qb + 1) * QB],
                                start=True,
                                stop=True,
                            )
                        nc.scalar.activation(
                            pt[:, 0:tn, :], sc[:, 0:tn, :], AF.Exp
                        )
                    # lagged PV + drain: runs while this group's exp is in
                    # flight
                    if prev_pv is not None:
                        emit_pv(*prev_pv)
                        prev_pv = None
                    if pend_drain is not None:
                        emit_drain(*pend_drain)
                        pend_drain = None
                    prev_pv = (t0, tn, pts, (pvA, pvB), (hA, hB), s0, s0 + sn - 1)
                pend_drain = ((pvA, pvB), (hA, hB), qb, sbi)

            emit_pv(*prev_pv)
            emit_drain(*pend_drain)
            while units:
                pop_unit()
            while epi:
                epi.popleft()()

    nc.compile()
    return nc


def kernel(x, context_tensor, mask, Wq, Wk, Wv, Wo, bo):
    from concourse.bass_utils import run_bass_kernel_spmd

    x = np.ascontiguousarray(np.asarray(x, dtype=np.float32))
    context_tensor = np.ascontiguousarray(np.asarray(context_tensor, dtype=np.float32))
    mask = np.asarray(mask)
    Wq = np.ascontiguousarray(np.asarray(Wq, dtype=np.float32))
    Wk = np.ascontiguousarray(np.asarray(Wk, dtype=np.float32))
    Wv = np.ascontiguousarray(np.asarray(Wv, dtype=np.float32))
    Wo = np.ascontiguousarray(np.asarray(Wo, dtype=np.float32))
    bo = np.ascontiguousarray(np.asarray(bo, dtype=np.float32))

    # host-side context compaction using the mask
    meffs = [int(mask[b].sum()) for b in range(B)]
    m_pad = max(M_PAD_MIN, ((max(meffs) + 127) // 128) * 128)
    ctx_c = np.zeros((B, m_pad, CONTEXT_DIM), dtype=np.float32)
    val = np.zeros((B, m_pad), dtype=np.float32)
    for b in range(B):
        idx = np.flatnonzero(mask[b])
        ctx_c[b, : len(idx)] = context_tensor[b, idx]
        val[b, : len(idx)] = 1.0

    if m_pad not in _compiled:
        _compiled[m_pad] = _build(m_pad)
    nc = _compiled[m_pad]

    rows_per_core = N // (NCORES // B)  # 1024
    in_maps = []
    for d in range(NCORES):
        b = d // (NCORES // B)
        r0 = (d % (NCORES // B)) * rows_per_core
        in_maps.append(
            {
                "xs": x[b, r0 : r0 + rows_per_core],
                "ctx": ctx_c[b],
                "valid": val[b],
                "Wq": Wq,
                "Wk": Wk,
                "Wv": Wv,
                "Wo": Wo,
                "bo": bo,
            }
        )

    res = run_bass_kernel_spmd(nc, in_maps, list(range(NCORES)))
    out = np.empty((B, N, QUERY_DIM), dtype=np.float32)
    for d in range(NCORES):
        b = d // (NCORES // B)
        r0 = (d % (NCORES // B)) * rows_per_core
        out[b, r0 : r0 + rows_per_core] = res.results[d]["out"]
    return out
